# revision 48
# baseline (speedup 1.0000x reference)
"""Trainium2 Bass kernel for GQA self-attention (non-causal, RoPE).

Reference computation (B=2, T=2048, C=2048, 16 q-heads, 4 kv-heads, d=128):
    q = x @ Wq.T ; k = x @ Wk.T ; v = x @ Wv.T
    q, k <- RoPE(q, k)
    att = softmax(q k^T / sqrt(d))        (no causal mask)
    out = att @ v ; y = out @ Wo.T
Sharding: 8 cores = DP(batch)=2 x TP(kv-head group)=4.  Core c handles batch
b=c//4, kv-group g=c%4 (q heads 4g..4g+3, kv head g).  Each core computes
y_partial = out_g @ Wo[:, 512g:512(g+1)].T; the host sums 4 partials/batch.

All on-chip data is bf16 (tolerance 2e-2; bf16 keeps us ~10x under it):
halves DMA bytes, doubles DVE throughput, same 1 cycle/row on the PE.

The limiting resources are the PE (~250us of matmul streaming) and ACT
(~133us of softmax exp that only the Activation engine can run).  Attention
is organized as one global software pipeline over the 16 (head, s-chunk)
groups: each group's 8 QK pair-matmuls + exps are emitted ~LOOK groups ahead
of its PV consumption, into a LOOK-deep ring of et buffers.  The QK/exp
stream starts inside phase A (once a chunk's k/q RoPE lands), so ACT's
otherwise-idle phase-A window absorbs the exp backlog and phase B runs at PE
rate instead of ACT rate.

  Phase A  q/k projections + RoPE (on DVE, partition-base-shifted muls),
           with QK/exp pairs pumped between projection matmul groups.
           v's projection is deferred so its PSUM bank can host the pre-run
           QK tiles; PSUM drains are split between ACT and Pool.
  v-pass   second sweep over x^T computing v (+ PE transposes to [t, d]);
           its x tiles are prefetched during phase A chunk 3.
  Phase B  per group: PV accumulation + rowsum (binary tree of bf16 adds on
           DVE/Pool + ONE ones-matmul) + 1/rowsum scaling, interleaved with
           QK/exp pumping for groups LOOK ahead.
  Phase C  y = outT^T @ Wo^T; PSUM drains on DVE/Pool, last tile's DMAs
           split across queues.
"""

import numpy as np

B = 2
T = 2048
C = 2048
HD = 128
N_HEAD = 16
N_KV = 4
ROPE_THETA = 10000.0
NCORES = 8
TP = 4  # kv-head groups
SCALE = 1.0 / float(np.sqrt(HD))

TCH = 512  # token chunk (matmul free dim)
NT = T // 128  # 16 token tiles of 128
NCH = T // TCH  # 4 token chunks
NKC = C // 128  # 16 contraction tiles
LOOK = 6  # et ring depth (groups of QK/exp in flight ahead of PV)

_CACHE = {}


def _build_nc():
    import concourse.bass as bass
    import concourse.mybir as mybir
    import concourse.tile as tile
    from concourse import bacc
    from concourse.masks import make_identity

    f32 = mybir.dt.float32
    bf16 = mybir.dt.bfloat16

    nc = bacc.Bacc(None)

    xT = nc.declare_dram_parameter("xT", [C, T], bf16, isOutput=False)
    wqT = nc.declare_dram_parameter("wqT", [C, 4 * HD], bf16, isOutput=False)
    wkT = nc.declare_dram_parameter("wkT", [C, HD], bf16, isOutput=False)
    wvT = nc.declare_dram_parameter("wvT", [C, HD], bf16, isOutput=False)
    woT = nc.declare_dram_parameter("woT", [4 * HD, C], bf16, isOutput=False)
    cosT = nc.declare_dram_parameter("cosT", [HD, T], bf16, isOutput=False)
    sinT = nc.declare_dram_parameter("sinT", [HD, T], bf16, isOutput=False)
    onesd = nc.declare_dram_parameter("ones", [128, 128], bf16, isOutput=False)
    permd = nc.declare_dram_parameter("perm", [128, 128], bf16, isOutput=False)
    y = nc.declare_dram_parameter("y", [T, C], bf16, isOutput=True)

    # attention groups in PV-consumption order: sc-major so early groups only
    # need early qT chunks
    GROUPS = [(sc, h) for sc in range(NCH) for h in range(4)]

    with tile.TileContext(nc) as tc:
        with (
            tc.tile_pool(name="persist", bufs=1) as persist,
            tc.tile_pool(name="small", bufs=1) as small,
            tc.tile_pool(name="epool", bufs=LOOK) as epool,
        ):
            # Persistent SBUF tensors
            qT_sb = persist.tile([128, 4, T], bf16)  # [d, qhead, t]
            kT_sb = persist.tile([128, T], bf16)  # [d, t]
            v_sb = persist.tile([128, NT, HD], bf16)  # [t%128, tblk, d]
            outT_sb = persist.tile([128, 4, T], bf16)  # [d, qhead, s]
            ones_sb = small.tile([128, 128], bf16)
            id_sb = small.tile([128, 128], bf16)
            perm_sb = small.tile([128, 128], bf16)

            # ---- global QK/exp pipeline state ----
            et_of = {}  # group idx -> et tile
            g_tp = [0] * len(GROUPS)  # next pair to emit per group
            pv_done = [0]  # groups fully PV-consumed (ring safety bound)

            # the pst pool in use for QK pairs (phase A uses a 1-buf pool
            # beside the projection banks; v-pass/phase B 2-buf pools)
            cur_pst = [None]

            def pump_qk(max_pairs, chunk_done, look=LOOK):
                """Emit up to max_pairs QK pair-matmuls + exps.  Groups are
                scanned in PV order but pairs are emitted by READINESS
                (a pair needs qT[sc] and kT[tp-chunk] RoPE'd), bounded by
                the et ring so the in-order PE can never deadlock."""
                emitted = 0
                hi = min(pv_done[0] + look, len(GROUPS))
                for gi in range(pv_done[0], hi):
                    sc, h = GROUPS[gi]
                    if sc > chunk_done:
                        break  # later groups need even later qT chunks
                    while emitted < max_pairs and g_tp[gi] < 8:
                        tp = g_tp[gi]
                        if tp // 2 > chunk_done:
                            break  # kT for this pair not yet scheduled
                        if tp == 0:
                            et_of[gi] = epool.tile(
                                [128, NT, TCH], bf16, tag="et", name=f"et{gi}"
                            )
                        et = et_of[gi]
                        pst = cur_pst[0].tile([128, 2, TCH], f32, tag="qkpst")
                        for u in range(2):
                            tt = 2 * tp + u
                            nc.tensor.matmul(
                                pst[:, u, :],
                                kT_sb[:, bass.ts(tt, 128)],
                                qT_sb[:, h, bass.ts(sc, TCH)],
                            )
                        nc.scalar.activation(
                            out=et[:, 2 * tp : 2 * tp + 2, :],
                            in_=pst[:],
                            func=mybir.ActivationFunctionType.Exp,
                            scale=SCALE,
                        )
                        g_tp[gi] += 1
                        emitted += 1
                    if emitted >= max_pairs:
                        return

            # ---------- Phase A: q/k projections + RoPE + QK pumping ----------
            with (
                tc.tile_pool(name="wA", bufs=1) as wA,
                tc.tile_pool(name="xload", bufs=5) as xload,
                tc.tile_pool(name="cossin", bufs=1) as cossin,
                tc.tile_pool(name="ropet", bufs=2) as ropet,
            ):
                cos_sb = cossin.tile([128, T], bf16)
                sin_sb = cossin.tile([128, T], bf16)
                wq_sb = wA.tile([128, NKC, 4 * HD], bf16)
                wk_sb = wA.tile([128, NKC, HD], bf16)
                wv_sb = wA.tile([128, NKC, HD], bf16)
                warm = wA.tile([128, 1], f32)

                # Startup: the DMA pool is one serial resource (~350B/ns), so
                # bytes queued ahead of the first x tile decide when the PE
                # starts.  Chunk0/group0 of x and wq go out in 2-k-slice
                # pieces; later weight groups follow just-in-time.
                def emit_wq(g4, split=False):
                    if split:
                        for lo, hi in ((0, 2), (2, 4)):
                            nc.scalar.dma_start(
                                out=wq_sb[:, 4 * g4 + lo : 4 * g4 + hi, :],
                                in_=wqT[
                                    g4 * 512 + lo * 128 : g4 * 512 + hi * 128, :
                                ].rearrange("(k p) d -> p k d", p=128),
                            )
                    else:
                        nc.scalar.dma_start(
                            out=wq_sb[:, 4 * g4 : 4 * g4 + 4, :],
                            in_=wqT[g4 * 512 : (g4 + 1) * 512, :].rearrange(
                                "(k p) d -> p k d", p=128
                            ),
                        )

                def emit_wk(g4):
                    nc.gpsimd.dma_start(
                        out=wk_sb[:, 4 * g4 : 4 * g4 + 4, :],
                        in_=wkT[g4 * 512 : (g4 + 1) * 512, :].rearrange(
                            "(k p) d -> p k d", p=128
                        ),
                    )

                def emit_wv(g4):
                    nc.gpsimd.dma_start(
                        out=wv_sb[:, 4 * g4 : 4 * g4 + 4, :],
                        in_=wvT[g4 * 512 : (g4 + 1) * 512, :].rearrange(
                            "(k p) d -> p k d", p=128
                        ),
                    )

                def emit_xt(n, g, split=False):
                    xt = xload.tile([128, 4, TCH], bf16, tag="xt", name="xt")
                    tsl = bass.ts(n, TCH)
                    if split:
                        for lo, hi in ((0, 2), (2, 4)):
                            nc.sync.dma_start(
                                out=xt[:, lo:hi, :],
                                in_=xT[
                                    g * 512 + lo * 128 : g * 512 + hi * 128, tsl
                                ].rearrange("(kk p) t -> p kk t", p=128),
                            )
                    else:
                        nc.sync.dma_start(
                            out=xt[:],
                            in_=xT[g * 512 : (g + 1) * 512, tsl].rearrange(
                                "(kk p) t -> p kk t", p=128
                            ),
                        )
                    return xt

                emit_wq(0, split=True)
                emit_wk(0)
                nc.gpsimd.dma_start(out=ones_sb[:], in_=onesd[:])
                nc.gpsimd.dma_start(out=perm_sb[:], in_=permd[:])
                nc.gpsimd.dma_start(out=cos_sb[:], in_=cosT[:])
                nc.gpsimd.dma_start(out=sin_sb[:], in_=sinT[:])
                make_identity(nc, id_sb[:])
                # warm the ACT exp table during the initial DMA wait
                nc.vector.memset(warm[:], 0.0)
                nc.scalar.activation(
                    out=warm[:], in_=warm[:],
                    func=mybir.ActivationFunctionType.Exp,
                )

                vpass_xt = []  # prefetched v-pass x tiles
                with (
                    tc.tile_pool(name="ppA", bufs=1, space="PSUM") as ppA,
                    tc.tile_pool(name="pstA", bufs=1, space="PSUM") as pstA,
                    tc.tile_pool(name="pqb", bufs=1, space="PSUM") as pqb,
                ):
                  cur_pst[0] = pstA
                  for n in range(NCH):
                    tsl = bass.ts(n, TCH)
                    pq = [
                        ppA.tile([128, TCH], f32, tag=f"pq{j}", name=f"pq{j}")
                        for j in range(4)
                    ]
                    pk = ppA.tile([128, TCH], f32, tag="pk")
                    # keep the PE fed while the previous chunk's PSUM banks
                    # drain on ACT/Pool
                    pump_qk(4, n - 1)
                    for g in range(4):  # 4-k-slice groups
                        if n == 0 and g > 0:
                            emit_wq(g)
                            emit_wk(g)
                        if n == 1 and g == 0:
                            for gv in range(4):
                                emit_wv(gv)
                        xt = emit_xt(n, g, split=(n == 0 and g == 0))
                        for u in range(4):
                            k = 4 * g + u
                            flags = dict(start=(k == 0), stop=(k == NKC - 1))
                            for j in range(4):
                                nc.tensor.matmul(
                                    pq[j][:],
                                    wq_sb[:, k, bass.ts(j, 128)],
                                    xt[:, u, :],
                                    **flags,
                                )
                            nc.tensor.matmul(
                                pk[:], wk_sb[:, k, :], xt[:, u, :], **flags
                            )
                        pump_qk(1, n - 1)
                    if n == NCH - 1:
                        # prefetch the first half of the v-pass x stream so
                        # the v-pass is not DMA-bound
                        for vn in range(2):
                            for vg in range(4):
                                vpass_xt.append(emit_xt(vn, vg))

                    # PSUM -> SBUF drains split across ACT and Pool; order
                    # matches next-chunk bank reuse, except k first on the
                    # last chunk (the QK stream needs kT immediately).
                    qa = {}
                    for j in range(5):
                        qa[j] = ropet.tile(
                            [128, TCH], bf16, tag=f"qa{j}", name=f"qa{j}"
                        )
                    act_order = (4, 0, 1) if n == NCH - 1 else (0, 4, 1)
                    for j in act_order:
                        src = pk if j == 4 else pq[j]
                        nc.scalar.copy(qa[j][:], src[:])
                    for j in (2, 3):
                        nc.vector.tensor_copy(qa[j][:], pq[j][:])

                    # RoPE: dst = qa*cos + perm(qa)*sin2, with the half
                    # rotation done as a PE permutation matmul (compute
                    # engines must keep all APs on the same partitions) and
                    # the rotate-half sign folded into the host sin2 table.
                    # k first: the QK stream needs kT before qT chunks.
                    jobs = [(4, kT_sb[:, tsl])] + [
                        (j, qT_sb[:, j, tsl]) for j in range(4)
                    ]
                    for j, dst in jobs:
                        qb = pqb.tile([128, TCH], f32, tag="qb")
                        nc.tensor.matmul(qb[:], perm_sb[:], qa[j][:])
                        t1 = ropet.tile([128, TCH], bf16, tag="t1")
                        nc.vector.tensor_mul(t1[:], qa[j][:], cos_sb[:, tsl])
                        t2 = ropet.tile([128, TCH], bf16, tag="t2")
                        nc.vector.tensor_mul(t2[:], qb[:], sin_sb[:, tsl])
                        nc.vector.tensor_add(dst, t1[:], t2[:])
                    pump_qk(4, n)

                # ---------------- v-pass ----------------
                # own PSUM layout: the projection banks are closed, so the
                # QK pump gets a double-buffered pst ring here
                with (
                    tc.tile_pool(name="ppV", bufs=1, space="PSUM") as ppV,
                    tc.tile_pool(name="pstV", bufs=2, space="PSUM") as pstV,
                    tc.tile_pool(name="ptvV", bufs=2, space="PSUM") as ptvV,
                ):
                  cur_pst[0] = pstV
                  pending_tr = []
                  for n in range(NCH):
                    pv = ppV.tile([128, TCH], f32, tag="pv")
                    if n + 1 >= 2 and n + 1 < NCH:
                        vpass_xt.extend(emit_xt(n + 1, g) for g in range(4))
                    xts = vpass_xt[4 * n : 4 * n + 4]
                    pump_qk(2, NCH - 1)
                    for g, x4 in enumerate(xts):
                        for u in range(4):
                            k = 4 * g + u
                            nc.tensor.matmul(
                                pv[:], wv_sb[:, k, :], x4[:, u, :],
                                start=(k == 0), stop=(k == NKC - 1),
                            )
                    pump_qk(2, NCH - 1)
                    vtmp = ropet.tile([128, TCH], bf16, tag="vtmp")
                    nc.scalar.copy(vtmp[:], pv[:])
                    for fn in pending_tr:
                        fn()
                    pending_tr = []

                    def make_tr(vtmp=vtmp, n=n):
                        def emit():
                            for tl in range(TCH // 128):
                                ptv = ptvV.tile([128, 128], bf16, tag="ptv")
                                nc.tensor.transpose(
                                    ptv[:], vtmp[:, bass.ts(tl, 128)], id_sb[:]
                                )
                                nc.vector.tensor_copy(
                                    v_sb[:, n * (TCH // 128) + tl, :], ptv[:]
                                )
                        return emit

                    pending_tr.append(make_tr())
                    pump_qk(4, NCH - 1)
                  for fn in pending_tr:
                    fn()

            # ---------------- Phase B: PV + rowsum + pumping ----------------
            with tc.tile_pool(name="wC", bufs=1) as wC:
                # prefetch Wo during attention (DMA pool is idle here)
                wo_sb = wC.tile([128, 4, C], bf16)
                nc.scalar.dma_start(
                    out=wo_sb[:], in_=woT[:].rearrange("(k p) m -> p k m", p=128)
                )

                with (
                    tc.tile_pool(name="rpool", bufs=2) as rpool,
                    tc.tile_pool(name="pstB", bufs=2, space="PSUM") as pstB,
                    tc.tile_pool(name="pacc", bufs=2, space="PSUM") as paccp,
                ):
                    cur_pst[0] = pstB
                    for gi, (sc, h) in enumerate(GROUPS):
                        ssl = bass.ts(sc, TCH)
                        et = et_of[gi]
                        pv_acc = paccp.tile([128, TCH], f32, tag="pvacc")
                        prs = paccp.tile([128, TCH], f32, tag="prs")
                        for tp in range(NT // 2):
                            pump_qk(1, NCH - 1)
                            for u in range(2):
                                tt = 2 * tp + u
                                nc.tensor.matmul(
                                    pv_acc[:],
                                    v_sb[:, tt, :],
                                    et[:, tt, :],
                                    start=(tt == 0),
                                    stop=(tt == NT - 1),
                                )
                            # rowsum: binary tree of bf16 adds, in place,
                            # spread over DVE and Pool
                            pair_eng = nc.gpsimd if tp in (0, 4) else nc.vector
                            pair_eng.tensor_add(
                                et[:, 2 * tp, :],
                                et[:, 2 * tp, :],
                                et[:, 2 * tp + 1, :],
                            )
                            if tp in (1, 3, 5, 7):
                                q0 = 4 * (tp // 2)
                                eng = nc.gpsimd if tp in (1, 5) else nc.vector
                                eng.tensor_add(
                                    et[:, q0, :], et[:, q0, :], et[:, 2 * tp, :]
                                )
                            if tp in (3, 7):
                                o0 = 8 * (tp // 4)
                                nc.vector.tensor_add(
                                    et[:, o0, :], et[:, o0, :], et[:, o0 + 4, :]
                                )
                        nc.vector.tensor_add(et[:, 0, :], et[:, 0, :], et[:, 8, :])
                        nc.tensor.matmul(prs[:], ones_sb[:], et[:, 0, :])
                        rec = rpool.tile([128, TCH], f32, tag="rec")
                        nc.vector.reciprocal(rec[:], prs[:])
                        nc.vector.tensor_mul(outT_sb[:, h, ssl], pv_acc[:], rec[:])
                        pv_done[0] = gi + 1

                # ---------------- Phase C: output projection ----------------
                with (
                    tc.tile_pool(name="ypool", bufs=4) as ypool,
                    tc.tile_pool(name="pyp", bufs=4, space="PSUM") as pyp,
                ):
                    for i in range(NT):
                        last = i == NT - 1
                        ysb = ypool.tile([128, C], bf16, tag="ysb")
                        for mc in range(NCH):
                            py = pyp.tile([128, TCH], f32, tag="py")
                            for kk in range(4):
                                nc.tensor.matmul(
                                    py[:],
                                    outT_sb[:, kk, bass.ts(i, 128)],
                                    wo_sb[:, kk, bass.ts(mc, TCH)],
                                    start=(kk == 0),
                                    stop=(kk == 3),
                                )
                            # PSUM drains off ACT (exp backlog may persist);
                            # last tile fans out over three engines so the
                            # closing DMA chain starts as early as possible
                            if mc % 2 == 0:
                                nc.vector.tensor_copy(ysb[:, bass.ts(mc, TCH)], py[:])
                            else:
                                nc.scalar.copy(ysb[:, bass.ts(mc, TCH)], py[:])
                            if last:
                                eng = (nc.sync, nc.scalar, nc.gpsimd, nc.sync)[mc]
                                eng.dma_start(
                                    out=y[i * 128 :, bass.ts(mc, TCH)],
                                    in_=ysb[:, bass.ts(mc, TCH)],
                                )
                        if not last:
                            nc.sync.dma_start(
                                out=y[i * 128 : (i + 1) * 128, :], in_=ysb[:]
                            )

    nc.compile()
    return nc


def _perm_matrix():
    # half-rotation permutation: out[d] = in[(d+64) % 128]; symmetric, so it
    # serves directly as the matmul lhsT.
    p = np.zeros((128, 128), dtype=np.float32)
    for i in range(128):
        p[i, (i + 64) % 128] = 1.0
    return p


def _rope_tables(start_pos):
    inv = (
        1.0
        / (ROPE_THETA ** (np.arange(0, HD, 2, dtype=np.float32) / np.float32(HD)))
    ).astype(np.float32)
    pos = np.arange(T, dtype=np.float32) + np.float32(start_pos)
    ang = pos[:, None] * inv[None, :]  # [T, 64]
    c = np.cos(ang, dtype=np.float32)
    s = np.sin(ang, dtype=np.float32)
    cosT = np.ascontiguousarray(np.concatenate([c, c], axis=1).T)  # [128, T]
    sin2 = np.concatenate([-s, s], axis=1)  # sign of rotate_half folded in
    sinT = np.ascontiguousarray(sin2.T)  # [128, T]
    return cosT, sinT


def kernel(x, Wq, Wk, Wv, Wo, start_pos):
    import os
    import sys

    if os.environ.get("JAX_PLATFORMS") == "cpu" and "jax" not in sys.modules:
        # the SPMD run needs the axon/neuron jax backend; drop a stray CPU
        # pin before jax initializes (no-op when jax is already loaded)
        del os.environ["JAX_PLATFORMS"]

    import ml_dtypes

    from concourse.bass_utils import run_bass_kernel_spmd

    bf = ml_dtypes.bfloat16

    if "nc" not in _CACHE:
        _CACHE["nc"] = _build_nc()
    nc = _CACHE["nc"]

    x = np.asarray(x, dtype=np.float32)
    Wq = np.asarray(Wq, dtype=np.float32)
    Wk = np.asarray(Wk, dtype=np.float32)
    Wv = np.asarray(Wv, dtype=np.float32)
    Wo = np.asarray(Wo, dtype=np.float32)
    cosT, sinT = _rope_tables(int(start_pos))
    cosT = cosT.astype(bf)
    sinT = sinT.astype(bf)
    xTs = [np.ascontiguousarray(x[b].T).astype(bf) for b in range(B)]
    ones = np.ones((128, 128), dtype=bf)
    perm = _perm_matrix().astype(bf)

    in_maps = []
    for c in range(NCORES):
        b, g = divmod(c, TP)
        in_maps.append(
            {
                "xT": xTs[b],
                "wqT": np.ascontiguousarray(Wq[512 * g : 512 * (g + 1), :].T).astype(bf),
                "wkT": np.ascontiguousarray(Wk[128 * g : 128 * (g + 1), :].T).astype(bf),
                "wvT": np.ascontiguousarray(Wv[128 * g : 128 * (g + 1), :].T).astype(bf),
                "woT": np.ascontiguousarray(Wo[:, 512 * g : 512 * (g + 1)].T).astype(bf),
                "cosT": cosT,
                "sinT": sinT,
                "ones": ones,
                "perm": perm,
            }
        )

    _CACHE["in_maps"] = in_maps
    res = run_bass_kernel_spmd(nc, in_maps, list(range(NCORES)))
    out = np.zeros((B, T, C), dtype=np.float32)
    for c in range(NCORES):
        out[c // TP] += np.asarray(res.results[c]["y"], dtype=np.float32)
    return out
